# revision 26
# baseline (speedup 1.0000x reference)
"""Trainium2 Bass kernel for causal self-attention (dense transformer block).

Full-input contract: kernel(**inputs) takes the complete tensors, shards
internally across 8 NeuronCores (tensor-parallel over heads x data-parallel
over batch: core = b*4 + g handles batch b, heads 4g..4g+3), runs one SPMD
Bass program, and gathers/reduces partial projections on the host.

Per-core pipeline (all-transposed layout, bf16 matmul operands with
fp32 PSUM accumulation; set DT_MM='f32r' for ~2e-4 accuracy at ~1.6x time):
  GEMM1a: qkT[f, t] = wqkvT.T-slices @ xT        (Q^T,K^T per head, [64, T])
  GEMM1b: V natural [t, 64] per head, augmented with a 64-wide ones block so
          the PV matmul emits softmax row-sums for free. Head parity flips
          the [V|1] / [1|V] order so O rows and sum rows land on the right
          partition halves for lane-aligned normalization.
  scores: ST[k, q] = K^T-slice.T @ Q^T-slice, two heads row-packed on the PE
          (base partitions 0/64 -> concurrent row groups), causal tiles only.
  exp:    one wide ACTIVATE per k-tile covering both heads' score banks
          (scale=1/8 folded in); fully-masked regions zero-filled on GPSIMD,
          diagonal strip masked with a 0/1 triangle (DVE multiply).
  PV:     O^T accumulates over k-tiles; the ones block gives 64 replicated
          softmax-denominator rows.
  norm:   DVE reciprocal + partition-shift DMA + DVE multiply.
  proj:   out_partial[t, :] = y_shard^T.T @ wpT  (contract 256 = 4 heads x 64)
"""

import numpy as np

N_HEAD = 16
N_EMBED = 1024
T_FULL = 2048
B_FULL = 2

_CACHE = {}


def _split_multiwait(nc, max_waits=1):
    """This walrus build allows only one sync-wait per instruction; move
    extras onto same-engine NoOps placed immediately before (engine queues
    are FIFO, so semantics are unchanged)."""
    import concourse.mybir as mybir

    counter = 0
    for f in nc.m.functions:
        for bb in f.blocks:
            changed = False
            new_list = []
            for ins in bb.instructions:
                si = ins.sync_info
                if si is not None and si.on_wait and len(si.on_wait) > max_waits:
                    waits = list(si.on_wait)
                    keep = waits[-max_waits:]
                    for w in waits[:-max_waits]:
                        counter += 1
                        nop = mybir.InstNoOp(
                            name=f"I-waitsplit-{counter}", ins=[], outs=[]
                        )
                        nop.engine = ins.engine
                        nop.sync_info = mybir.SyncInfo(on_wait=[w], on_update=[])
                        nc.register_instruction(nop)
                        new_list.append(nop)
                    ins.sync_info = mybir.SyncInfo(
                        on_wait=keep, on_update=list(si.on_update or [])
                    )
                    changed = True
                new_list.append(ins)
            if changed:
                bb.instructions = new_list
    return counter


def build_nc(T=T_FULL, C=N_EMBED, QSPAN=512, CW=512, dt_mm="bf16"):
    """Build the SPMD Bass program for one core's shard (4 heads, one batch)."""
    import contextlib

    import concourse.bass as bass
    import concourse.mybir as mybir
    import concourse.tile as tile

    F32 = mybir.dt.float32
    F32R = mybir.dt.bfloat16 if dt_mm == "bf16" else mybir.dt.float32r
    DT_IN = mybir.dt.bfloat16 if dt_mm == "bf16" else mybir.dt.float32
    NCT = C // 128          # c-tiles (contraction of GEMM1)
    NKT = T // 128          # k-tiles
    QSPAN = min(QSPAN, T)   # q processed per pass
    NQP = T // QSPAN        # passes
    CW = min(CW, QSPAN)     # matmul/chunk width along q
    NCH = QSPAN // CW       # chunks per pass
    NTT = T // 128          # t-tiles for GEMM1b / proj
    KPQ = QSPAN // 128      # k-tiles per q-span
    W1 = min(1024, T)       # GEMM1a psum tile width along t
    NTH = T // W1

    nc = bass.Bass("TRN2", target_bir_lowering=False, debug=False, num_devices=8)
    xT_d = nc.dram_tensor("xT", [C, T], DT_IN, kind="ExternalInput")
    wq_d = nc.dram_tensor("wqkvT", [C, 768], DT_IN, kind="ExternalInput")
    wp_d = nc.dram_tensor("wpT", [256, C], DT_IN, kind="ExternalInput")
    bqk_d = nc.dram_tensor("bqk", [128, 4], F32, kind="ExternalInput")
    yp_d = nc.dram_tensor("yp", [T, C], F32, kind="ExternalOutput")

    Exp = mybir.ActivationFunctionType.Exp

    with tile.TileContext(nc) as tc, nc.allow_low_precision(
        reason="fp32r compute path; accumulation stays fp32 in PSUM"
    ):
        with contextlib.ExitStack() as ctx:
            persist = ctx.enter_context(tc.tile_pool(name="persist", bufs=1))
            # Persistent SBUF tensors
            qkT = persist.tile([128, 4, T], F32R)        # f-tiles: q01 q23 k01 k23
            vv = persist.tile([128, NKT, 4, 128], F32R)  # V'' per k-tile per head
            otsb = persist.tile([128, 2, T], F32R)       # y_shard^T (pair-major)
            wpsb = persist.tile([128, 2, C], F32R)
            mask = persist.tile([128, 640], DT_IN)       # [0..512)=0, [512..640)=tri
            bqk = persist.tile([128, 4], F32)

            wp_src = wp_d.ap().rearrange("(s p) c -> p s c", p=128)
            if dt_mm != "bf16":
                wp_src = wp_src.bitcast(F32R)
            nc.sync.dma_start(wpsb[:], wp_src)
            nc.sync.dma_start(bqk[:], bqk_d.ap())
            # mask: zeros then keep-1.0 where q >= k (lower-left triangle)
            nc.gpsimd.memset(mask[:, 0:512], 0.0)
            nc.gpsimd.memset(mask[:, 512:640], 1.0)
            nc.gpsimd.affine_select(
                out=mask[:, 512:640], in_=mask[:, 512:640],
                pattern=[[1, 128]], channel_multiplier=-1, base=0,
                compare_op=mybir.AluOpType.is_ge, fill=0.0,
            )
            # ones blocks of V'': even heads [V|1], odd heads [1|V].
            # memset can't emit float32r, so build 1.0 = 0.0 + 1.0 on the DVE
            # (a valid fp32r producer) from the zeroed mask region.
            for j in range(NKT):
                for hh in range(4):
                    half = slice(64, 128) if hh % 2 == 0 else slice(0, 64)
                    nc.vector.tensor_scalar_add(vv[:, j, hh, half], mask[:, 0:64], 1.0)

            with contextlib.ExitStack() as g1ctx:
                inp = g1ctx.enter_context(tc.tile_pool(name="inp", bufs=1))
                g1ps = g1ctx.enter_context(tc.tile_pool(name="g1ps", bufs=2, space="PSUM"))
                xsrc = xT_d.ap().rearrange("(a p) t -> p a t", p=128)
                wsrc = wq_d.ap().rearrange("(a p) f -> p a f", p=128)
                if dt_mm != "bf16":
                    xsrc = xsrc.bitcast(F32R)
                    wsrc = wsrc.bitcast(F32R)
                xT = []
                wq = []
                for c in range(NCT):
                    wqc = inp.tile([128, 768], F32R, tag=f"wq{c}", name=f"wq{c}")
                    xTc = inp.tile([128, T], F32R, tag=f"xT{c}", name=f"xT{c}")
                    nc.sync.dma_start(wqc[:], wsrc[:, c, :])
                    for th0 in range(0, T, 1024):
                        w0 = min(1024, T - th0)
                        nc.sync.dma_start(
                            xTc[:, th0:th0 + w0], xsrc[:, c, th0:th0 + w0]
                        )
                    wq.append(wqc)
                    xT.append(xTc)

                # GEMM1a: qkT[f, :] for the 4 q/k f-tiles. The 128-deep
                # c-contraction runs as two 64-row halves into separate PSUM
                # banks (concurrent row groups, weight loads overlap), summed
                # on the DVE during the bias-add copy.
                for th in range(NTH):
                    for f in range(4):
                        ps = g1ps.tile([128, W1], F32, tag="g1qk", name=f"qk{f}_{th}")
                        for c in range(NCT):
                            for ch in range(W1 // 512):
                                nc.tensor.matmul(
                                    ps[:, ch * 512:(ch + 1) * 512],
                                    wq[c][:, f * 128:(f + 1) * 128],
                                    xT[c][:, th * W1 + ch * 512: th * W1 + (ch + 1) * 512],
                                    start=(c == 0), stop=(c == NCT - 1),
                                )
                        nc.vector.tensor_scalar_add(
                            qkT[:, f, th * W1: (th + 1) * W1], ps[:], bqk[:, f:f + 1]
                        )

                # GEMM1b: V natural, strided into V'' (parity-dependent half)
                for tt in range(NTT):
                    psv = g1ps.tile([128, 256], F32, tag="g1v", name=f"v{tt}")
                    for c in range(NCT):
                        nc.tensor.matmul(
                            psv[:],
                            xT[c][:, tt * 128:(tt + 1) * 128],
                            wq[c][:, 512:768],
                            start=(c == 0), stop=(c == NCT - 1),
                        )
                    for hh in range(4):
                        dst = slice(0, 64) if hh % 2 == 0 else slice(64, 128)
                        nc.vector.tensor_copy(
                            vv[:, tt, hh, dst], psv[:, 64 * hh:64 * hh + 64]
                        )

            with contextlib.ExitStack() as actx:
                apool = actx.enter_context(tc.tile_pool(name="attn", bufs=2))
                ptpool = actx.enter_context(tc.tile_pool(name="pt", bufs=3))
                aps = actx.enter_context(tc.tile_pool(name="aps", bufs=2, space="PSUM"))

                for p in range(2):
                    for qh in range(NQP):
                        q0 = qh * QSPAN
                        # one PSUM tile per accumulation chunk: the matmul
                        # start=True has_written clear acts bank-wide, so two
                        # interleaved accumulation groups must not share a bank
                        ot = [
                            [
                                aps.tile([128, CW], F32, tag=f"ot{h}_{ch}",
                                         name=f"ot{h}_{ch}_{p}_{qh}")
                                for ch in range(NCH)
                            ]
                            for h in range(2)
                        ]
                        kmax = KPQ * (qh + 1)
                        # software pipeline: emit each PV one iteration after
                        # its exp, so scores_{j+1} sits ahead of PV_j in the
                        # PE FIFO and the PE never head-of-line blocks on ACT
                        pending = None
                        def emit_pv(j, ptp):
                            for ch in range(NCH):
                                jlast = min(kmax - 1, (q0 + CW * (ch + 1) - 1) // 128)
                                for h in range(2):
                                    nc.tensor.matmul(
                                        ot[h][ch][:],
                                        vv[:, j, 2 * p + h, :],
                                        ptp[:, h * QSPAN + ch * CW: h * QSPAN + (ch + 1) * CW],
                                        start=(j == 0), stop=(j == jlast),
                                    )
                        for j in range(kmax):
                            col0 = max(0, 128 * j - q0)
                            stp = aps.tile([128, 2 * QSPAN], F32, tag="stp",
                                           name=f"stp_{p}_{qh}_{j}")
                            for h in range(2):
                                nc.tensor.matmul(
                                    stp[:, h * QSPAN: h * QSPAN + QSPAN],
                                    qkT[64 * h:64 * h + 64, 2 + p, 128 * j:128 * (j + 1)],
                                    qkT[64 * h:64 * h + 64, p, q0:q0 + QSPAN],
                                    start=True, stop=True,
                                )
                            ptp = ptpool.tile([128, 2 * QSPAN], F32R, tag="ptp",
                                              name=f"ptp_{p}_{qh}_{j}")
                            strip = 128 * j >= q0
                            if col0 == 0:
                                nc.scalar.activation(
                                    ptp[:], stp[:], Exp, scale=0.125,
                                )
                            else:
                                nc.scalar.activation(
                                    ptp[:].rearrange("p (h q) -> p h q", h=2)[:, :, col0:QSPAN],
                                    stp[:].rearrange("p (h q) -> p h q", h=2)[:, :, col0:QSPAN],
                                    Exp, scale=0.125,
                                )
                            for h in range(2):
                                if strip:
                                    if col0 > 0:
                                        nc.gpsimd.memset(
                                            ptp[:, h * QSPAN: h * QSPAN + col0], 0.0
                                        )
                                    nc.vector.tensor_mul(
                                        ptp[:, h * QSPAN + col0: h * QSPAN + col0 + 128],
                                        ptp[:, h * QSPAN + col0: h * QSPAN + col0 + 128],
                                        mask[:, 512:640],
                                    )
                            if pending is not None:
                                emit_pv(*pending)
                            pending = (j, ptp)
                        emit_pv(*pending)
                        rc = apool.tile([128, QSPAN], F32, tag="rc", name=f"rc_{p}_{qh}")
                        rc2 = apool.tile([128, QSPAN], F32, tag="rc2", name=f"rc2_{p}_{qh}")
                        rc3 = apool.tile([128, QSPAN], F32, tag="rc3", name=f"rc3_{p}_{qh}")
                        for ch in range(NCH):
                            sl = slice(ch * CW, (ch + 1) * CW)
                            # both heads' replicated denominators sit on
                            # complementary halves (V'' parity trick): gather,
                            # one combined reciprocal, swap halves via DMA,
                            # then scale each head's O rows
                            nc.vector.tensor_copy(rc[64:128, sl], ot[0][ch][64:128, :])
                            nc.vector.tensor_copy(rc[0:64, sl], ot[1][ch][0:64, :])
                            nc.vector.reciprocal(rc2[:, sl], rc[:, sl])
                            nc.sync.dma_start(rc3[0:64, sl], rc2[64:128, sl])
                            nc.sync.dma_start(rc3[64:128, sl], rc2[0:64, sl])
                            nc.vector.tensor_mul(
                                otsb[0:64, p, q0 + ch * CW: q0 + (ch + 1) * CW],
                                ot[0][ch][0:64, :], rc3[0:64, sl],
                            )
                            nc.vector.tensor_mul(
                                otsb[64:128, p, q0 + ch * CW: q0 + (ch + 1) * CW],
                                ot[1][ch][64:128, :], rc3[64:128, sl],
                            )
            with contextlib.ExitStack() as pctx:
                spool = pctx.enter_context(tc.tile_pool(name="stage", bufs=3))
                pps = pctx.enter_context(tc.tile_pool(name="pps", bufs=4, space="PSUM"))
                for tt in range(NTT):
                    stage = spool.tile([128, C], F32, tag="out")
                    for ch in range(C // 512 if C >= 512 else 1):
                        w2 = min(512, C)
                        pp = pps.tile([128, w2], F32, tag="pj", name=f"pj{tt}_{ch}")
                        for p in range(2):
                            nc.tensor.matmul(
                                pp[:],
                                otsb[:, p, tt * 128:(tt + 1) * 128],
                                wpsb[:, p, ch * 512: ch * 512 + w2],
                                start=(p == 0), stop=(p == 1),
                            )
                        nc.scalar.copy(stage[:, ch * 512: ch * 512 + w2], pp[:])
                    nc.sync.dma_start(yp_d.ap()[tt * 128:(tt + 1) * 128, :], stage[:])

    _split_multiwait(nc)
    return nc


def _shard_inputs(x, w_attn, b_attn, w_proj, T, C, dt_mm="bf16"):
    """Host-side shard prep. Returns per-core input dicts (8 cores)."""
    if dt_mm == "bf16":
        import ml_dtypes
        cast = lambda a: np.ascontiguousarray(a).astype(ml_dtypes.bfloat16)
    else:
        cast = np.ascontiguousarray
    in_maps = []
    xTs = [cast(x[b].T) for b in range(x.shape[0])]
    for core in range(8):
        b, g = core // 4, core % 4
        rows = np.r_[256 * g:256 * g + 256,
                     C + 256 * g:C + 256 * g + 256,
                     2 * C + 256 * g:2 * C + 256 * g + 256]
        wqkvT = cast(w_attn[rows, :].T)
        wpT = cast(w_proj[:, 256 * g:256 * g + 256].T)
        bqk = np.ascontiguousarray(b_attn[rows[:512]].reshape(4, 128).T)
        in_maps.append({"xT": xTs[b], "wqkvT": wqkvT, "wpT": wpT, "bqk": bqk})
    return in_maps


def _run(nc, in_maps, trace=False):
    from concourse.bass_utils import run_bass_kernel_spmd
    return run_bass_kernel_spmd(nc, in_maps, core_ids=list(range(8)), trace=trace)


DT_MM = "bf16"


def kernel(x, w_attn, b_attn, w_proj, b_proj, _trace=False, _results_out=None):
    B, T, C = x.shape
    if "nc" not in _CACHE:
        _CACHE["nc"] = build_nc(T, C, dt_mm=DT_MM)
    nc = _CACHE["nc"]
    in_maps = _shard_inputs(
        np.asarray(x), np.asarray(w_attn), np.asarray(b_attn), np.asarray(w_proj),
        T, C, dt_mm=DT_MM,
    )
    res = _run(nc, in_maps, trace=_trace)
    if _results_out is not None:
        _results_out.append(res)
    # Host gather: sum the 4 tensor-parallel partials per batch, add the exact
    # V-bias term (softmax rows sum to 1) and the projection bias.
    vbias_term = (
        np.asarray(b_attn)[2 * C:3 * C].astype(np.float32) @ np.asarray(w_proj).T
    )
    out = np.empty((B, T, C), dtype=np.float32)
    for b in range(B):
        acc = res.results[4 * b]["yp"].astype(np.float32).copy()
        for g in range(1, 4):
            acc += res.results[4 * b + g]["yp"]
        out[b] = acc + vbias_term + np.asarray(b_proj)
    return out


# revision 27
# speedup vs baseline: 1.0216x; 1.0216x over previous
"""Trainium2 Bass kernel for causal self-attention (dense transformer block).

Full-input contract: kernel(**inputs) takes the complete tensors, shards
internally across 8 NeuronCores (tensor-parallel over heads x data-parallel
over batch: core = b*4 + g handles batch b, heads 4g..4g+3), runs one SPMD
Bass program, and gathers/reduces partial projections on the host.

Per-core pipeline (all-transposed layout, bf16 matmul operands with
fp32 PSUM accumulation; set DT_MM='f32r' for ~2e-4 accuracy at ~1.6x time):
  GEMM1a: qkT[f, t] = wqkvT.T-slices @ xT        (Q^T,K^T per head, [64, T])
  GEMM1b: V natural [t, 64] per head, augmented with a 64-wide ones block so
          the PV matmul emits softmax row-sums for free. Head parity flips
          the [V|1] / [1|V] order so O rows and sum rows land on the right
          partition halves for lane-aligned normalization.
  scores: ST[k, q] = K^T-slice.T @ Q^T-slice, two heads row-packed on the PE
          (base partitions 0/64 -> concurrent row groups), causal tiles only.
  exp:    one wide ACTIVATE per k-tile covering both heads' score banks
          (scale=1/8 folded in); fully-masked regions zero-filled on GPSIMD,
          diagonal strip masked with a 0/1 triangle (DVE multiply).
  PV:     O^T accumulates over k-tiles; the ones block gives 64 replicated
          softmax-denominator rows.
  norm:   DVE reciprocal + partition-shift DMA + DVE multiply.
  proj:   out_partial[t, :] = y_shard^T.T @ wpT  (contract 256 = 4 heads x 64)
"""

import numpy as np

N_HEAD = 16
N_EMBED = 1024
T_FULL = 2048
B_FULL = 2

_CACHE = {}


def _split_multiwait(nc, max_waits=1):
    """This walrus build allows only one sync-wait per instruction; move
    extras onto same-engine NoOps placed immediately before (engine queues
    are FIFO, so semantics are unchanged)."""
    import concourse.mybir as mybir

    counter = 0
    for f in nc.m.functions:
        for bb in f.blocks:
            changed = False
            new_list = []
            for ins in bb.instructions:
                si = ins.sync_info
                if si is not None and si.on_wait and len(si.on_wait) > max_waits:
                    waits = list(si.on_wait)
                    keep = waits[-max_waits:]
                    for w in waits[:-max_waits]:
                        counter += 1
                        nop = mybir.InstNoOp(
                            name=f"I-waitsplit-{counter}", ins=[], outs=[]
                        )
                        nop.engine = ins.engine
                        nop.sync_info = mybir.SyncInfo(on_wait=[w], on_update=[])
                        nc.register_instruction(nop)
                        new_list.append(nop)
                    ins.sync_info = mybir.SyncInfo(
                        on_wait=keep, on_update=list(si.on_update or [])
                    )
                    changed = True
                new_list.append(ins)
            if changed:
                bb.instructions = new_list
    return counter


def build_nc(T=T_FULL, C=N_EMBED, QSPAN=512, CW=512, dt_mm="bf16"):
    """Build the SPMD Bass program for one core's shard (4 heads, one batch)."""
    import contextlib

    import concourse.bass as bass
    import concourse.mybir as mybir
    import concourse.tile as tile

    F32 = mybir.dt.float32
    F32R = mybir.dt.bfloat16 if dt_mm == "bf16" else mybir.dt.float32r
    DT_IN = mybir.dt.bfloat16 if dt_mm == "bf16" else mybir.dt.float32
    NCT = C // 128          # c-tiles (contraction of GEMM1)
    NKT = T // 128          # k-tiles
    QSPAN = min(QSPAN, T)   # q processed per pass
    NQP = T // QSPAN        # passes
    CW = min(CW, QSPAN)     # matmul/chunk width along q
    NCH = QSPAN // CW       # chunks per pass
    NTT = T // 128          # t-tiles for GEMM1b / proj
    KPQ = QSPAN // 128      # k-tiles per q-span
    W1 = min(1024, T)       # GEMM1a psum tile width along t
    NTH = T // W1

    nc = bass.Bass("TRN2", target_bir_lowering=False, debug=False, num_devices=8)
    xT_d = nc.dram_tensor("xT", [C, T], DT_IN, kind="ExternalInput")
    wq_d = nc.dram_tensor("wqkvT", [C, 768], DT_IN, kind="ExternalInput")
    wp_d = nc.dram_tensor("wpT", [256, C], DT_IN, kind="ExternalInput")
    bqk_d = nc.dram_tensor("bqk", [128, 4], F32, kind="ExternalInput")
    yp_d = nc.dram_tensor("yp", [T, C], DT_IN, kind="ExternalOutput")

    Exp = mybir.ActivationFunctionType.Exp

    with tile.TileContext(nc) as tc, nc.allow_low_precision(
        reason="fp32r compute path; accumulation stays fp32 in PSUM"
    ):
        with contextlib.ExitStack() as ctx:
            persist = ctx.enter_context(tc.tile_pool(name="persist", bufs=1))
            # Persistent SBUF tensors
            qkT = persist.tile([128, 4, T], F32R)        # f-tiles: q01 q23 k01 k23
            vv = persist.tile([128, NKT, 4, 128], F32R)  # V'' per k-tile per head
            otsb = persist.tile([128, 2, T], F32R)       # y_shard^T (pair-major)
            wpsb = persist.tile([128, 2, C], F32R)
            mask = persist.tile([128, 640], DT_IN)       # [0..512)=0, [512..640)=tri
            bqk = persist.tile([128, 4], F32)

            wp_src = wp_d.ap().rearrange("(s p) c -> p s c", p=128)
            if dt_mm != "bf16":
                wp_src = wp_src.bitcast(F32R)
            nc.sync.dma_start(wpsb[:], wp_src)
            nc.sync.dma_start(bqk[:], bqk_d.ap())
            # mask: zeros then keep-1.0 where q >= k (lower-left triangle)
            nc.gpsimd.memset(mask[:, 0:512], 0.0)
            nc.gpsimd.memset(mask[:, 512:640], 1.0)
            nc.gpsimd.affine_select(
                out=mask[:, 512:640], in_=mask[:, 512:640],
                pattern=[[1, 128]], channel_multiplier=-1, base=0,
                compare_op=mybir.AluOpType.is_ge, fill=0.0,
            )
            # ones blocks of V'': even heads [V|1], odd heads [1|V].
            # memset can't emit float32r, so build 1.0 = 0.0 + 1.0 on the DVE
            # (a valid fp32r producer) from the zeroed mask region.
            for j in range(NKT):
                for hh in range(4):
                    half = slice(64, 128) if hh % 2 == 0 else slice(0, 64)
                    nc.vector.tensor_scalar_add(vv[:, j, hh, half], mask[:, 0:64], 1.0)

            with contextlib.ExitStack() as g1ctx:
                inp = g1ctx.enter_context(tc.tile_pool(name="inp", bufs=1))
                g1ps = g1ctx.enter_context(tc.tile_pool(name="g1ps", bufs=2, space="PSUM"))
                xsrc = xT_d.ap().rearrange("(a p) t -> p a t", p=128)
                wsrc = wq_d.ap().rearrange("(a p) f -> p a f", p=128)
                if dt_mm != "bf16":
                    xsrc = xsrc.bitcast(F32R)
                    wsrc = wsrc.bitcast(F32R)
                xT = []
                wq = []
                for c in range(NCT):
                    wqc = inp.tile([128, 768], F32R, tag=f"wq{c}", name=f"wq{c}")
                    xTc = inp.tile([128, T], F32R, tag=f"xT{c}", name=f"xT{c}")
                    nc.sync.dma_start(wqc[:], wsrc[:, c, :])
                    for th0 in range(0, T, 1024):
                        w0 = min(1024, T - th0)
                        nc.sync.dma_start(
                            xTc[:, th0:th0 + w0], xsrc[:, c, th0:th0 + w0]
                        )
                    wq.append(wqc)
                    xT.append(xTc)

                # GEMM1a: qkT[f, :] for the 4 q/k f-tiles. The 128-deep
                # c-contraction runs as two 64-row halves into separate PSUM
                # banks (concurrent row groups, weight loads overlap), summed
                # on the DVE during the bias-add copy.
                for th in range(NTH):
                    for f in range(4):
                        ps = g1ps.tile([128, W1], F32, tag="g1qk", name=f"qk{f}_{th}")
                        for c in range(NCT):
                            for ch in range(W1 // 512):
                                nc.tensor.matmul(
                                    ps[:, ch * 512:(ch + 1) * 512],
                                    wq[c][:, f * 128:(f + 1) * 128],
                                    xT[c][:, th * W1 + ch * 512: th * W1 + (ch + 1) * 512],
                                    start=(c == 0), stop=(c == NCT - 1),
                                )
                        nc.vector.tensor_scalar_add(
                            qkT[:, f, th * W1: (th + 1) * W1], ps[:], bqk[:, f:f + 1]
                        )

                # GEMM1b: V natural, strided into V'' (parity-dependent half)
                for tt in range(NTT):
                    psv = g1ps.tile([128, 256], F32, tag="g1v", name=f"v{tt}")
                    for c in range(NCT):
                        nc.tensor.matmul(
                            psv[:],
                            xT[c][:, tt * 128:(tt + 1) * 128],
                            wq[c][:, 512:768],
                            start=(c == 0), stop=(c == NCT - 1),
                        )
                    for hh in range(4):
                        dst = slice(0, 64) if hh % 2 == 0 else slice(64, 128)
                        nc.vector.tensor_copy(
                            vv[:, tt, hh, dst], psv[:, 64 * hh:64 * hh + 64]
                        )

            with contextlib.ExitStack() as actx:
                apool = actx.enter_context(tc.tile_pool(name="attn", bufs=2))
                ptpool = actx.enter_context(tc.tile_pool(name="pt", bufs=3))
                aps = actx.enter_context(tc.tile_pool(name="aps", bufs=2, space="PSUM"))

                for p in range(2):
                    for qh in range(NQP):
                        q0 = qh * QSPAN
                        # one PSUM tile per accumulation chunk: the matmul
                        # start=True has_written clear acts bank-wide, so two
                        # interleaved accumulation groups must not share a bank
                        ot = [
                            [
                                aps.tile([128, CW], F32, tag=f"ot{h}_{ch}",
                                         name=f"ot{h}_{ch}_{p}_{qh}")
                                for ch in range(NCH)
                            ]
                            for h in range(2)
                        ]
                        kmax = KPQ * (qh + 1)
                        # software pipeline: emit each PV one iteration after
                        # its exp, so scores_{j+1} sits ahead of PV_j in the
                        # PE FIFO and the PE never head-of-line blocks on ACT
                        pending = None
                        def emit_pv(j, ptp):
                            for ch in range(NCH):
                                jlast = min(kmax - 1, (q0 + CW * (ch + 1) - 1) // 128)
                                for h in range(2):
                                    nc.tensor.matmul(
                                        ot[h][ch][:],
                                        vv[:, j, 2 * p + h, :],
                                        ptp[:, h * QSPAN + ch * CW: h * QSPAN + (ch + 1) * CW],
                                        start=(j == 0), stop=(j == jlast),
                                    )
                        for j in range(kmax):
                            col0 = max(0, 128 * j - q0)
                            stp = aps.tile([128, 2 * QSPAN], F32, tag="stp",
                                           name=f"stp_{p}_{qh}_{j}")
                            for h in range(2):
                                nc.tensor.matmul(
                                    stp[:, h * QSPAN: h * QSPAN + QSPAN],
                                    qkT[64 * h:64 * h + 64, 2 + p, 128 * j:128 * (j + 1)],
                                    qkT[64 * h:64 * h + 64, p, q0:q0 + QSPAN],
                                    start=True, stop=True,
                                )
                            ptp = ptpool.tile([128, 2 * QSPAN], F32R, tag="ptp",
                                              name=f"ptp_{p}_{qh}_{j}")
                            strip = 128 * j >= q0
                            if col0 == 0:
                                nc.scalar.activation(
                                    ptp[:], stp[:], Exp, scale=0.125,
                                )
                            else:
                                nc.scalar.activation(
                                    ptp[:].rearrange("p (h q) -> p h q", h=2)[:, :, col0:QSPAN],
                                    stp[:].rearrange("p (h q) -> p h q", h=2)[:, :, col0:QSPAN],
                                    Exp, scale=0.125,
                                )
                            for h in range(2):
                                if strip:
                                    if col0 > 0:
                                        nc.gpsimd.memset(
                                            ptp[:, h * QSPAN: h * QSPAN + col0], 0.0
                                        )
                                    nc.vector.tensor_mul(
                                        ptp[:, h * QSPAN + col0: h * QSPAN + col0 + 128],
                                        ptp[:, h * QSPAN + col0: h * QSPAN + col0 + 128],
                                        mask[:, 512:640],
                                    )
                            if pending is not None:
                                emit_pv(*pending)
                            pending = (j, ptp)
                        emit_pv(*pending)
                        rc = apool.tile([128, QSPAN], F32, tag="rc", name=f"rc_{p}_{qh}")
                        rc2 = apool.tile([128, QSPAN], F32, tag="rc2", name=f"rc2_{p}_{qh}")
                        rc3 = apool.tile([128, QSPAN], F32, tag="rc3", name=f"rc3_{p}_{qh}")
                        for ch in range(NCH):
                            sl = slice(ch * CW, (ch + 1) * CW)
                            # both heads' replicated denominators sit on
                            # complementary halves (V'' parity trick): gather,
                            # one combined reciprocal, swap halves via DMA,
                            # then scale each head's O rows
                            nc.vector.tensor_copy(rc[64:128, sl], ot[0][ch][64:128, :])
                            nc.vector.tensor_copy(rc[0:64, sl], ot[1][ch][0:64, :])
                            nc.vector.reciprocal(rc2[:, sl], rc[:, sl])
                            nc.sync.dma_start(rc3[0:64, sl], rc2[64:128, sl])
                            nc.sync.dma_start(rc3[64:128, sl], rc2[0:64, sl])
                            nc.vector.tensor_mul(
                                otsb[0:64, p, q0 + ch * CW: q0 + (ch + 1) * CW],
                                ot[0][ch][0:64, :], rc3[0:64, sl],
                            )
                            nc.vector.tensor_mul(
                                otsb[64:128, p, q0 + ch * CW: q0 + (ch + 1) * CW],
                                ot[1][ch][64:128, :], rc3[64:128, sl],
                            )
            with contextlib.ExitStack() as pctx:
                spool = pctx.enter_context(tc.tile_pool(name="stage", bufs=3))
                pps = pctx.enter_context(tc.tile_pool(name="pps", bufs=4, space="PSUM"))
                for tt in range(NTT):
                    stage = spool.tile([128, C], DT_IN, tag="out")
                    for ch in range(C // 512 if C >= 512 else 1):
                        w2 = min(512, C)
                        pp = pps.tile([128, w2], F32, tag="pj", name=f"pj{tt}_{ch}")
                        for p in range(2):
                            nc.tensor.matmul(
                                pp[:],
                                otsb[:, p, tt * 128:(tt + 1) * 128],
                                wpsb[:, p, ch * 512: ch * 512 + w2],
                                start=(p == 0), stop=(p == 1),
                            )
                        nc.scalar.copy(stage[:, ch * 512: ch * 512 + w2], pp[:])
                    nc.sync.dma_start(yp_d.ap()[tt * 128:(tt + 1) * 128, :], stage[:])

    _split_multiwait(nc)
    return nc


def _shard_inputs(x, w_attn, b_attn, w_proj, T, C, dt_mm="bf16"):
    """Host-side shard prep. Returns per-core input dicts (8 cores)."""
    if dt_mm == "bf16":
        import ml_dtypes
        cast = lambda a: np.ascontiguousarray(a).astype(ml_dtypes.bfloat16)
    else:
        cast = np.ascontiguousarray
    in_maps = []
    xTs = [cast(x[b].T) for b in range(x.shape[0])]
    for core in range(8):
        b, g = core // 4, core % 4
        rows = np.r_[256 * g:256 * g + 256,
                     C + 256 * g:C + 256 * g + 256,
                     2 * C + 256 * g:2 * C + 256 * g + 256]
        wqkvT = cast(w_attn[rows, :].T)
        wpT = cast(w_proj[:, 256 * g:256 * g + 256].T)
        bqk = np.ascontiguousarray(b_attn[rows[:512]].reshape(4, 128).T)
        in_maps.append({"xT": xTs[b], "wqkvT": wqkvT, "wpT": wpT, "bqk": bqk})
    return in_maps


def _run(nc, in_maps, trace=False):
    from concourse.bass_utils import run_bass_kernel_spmd
    return run_bass_kernel_spmd(nc, in_maps, core_ids=list(range(8)), trace=trace)


DT_MM = "bf16"


def kernel(x, w_attn, b_attn, w_proj, b_proj, _trace=False, _results_out=None):
    B, T, C = x.shape
    if "nc" not in _CACHE:
        _CACHE["nc"] = build_nc(T, C, dt_mm=DT_MM)
    nc = _CACHE["nc"]
    in_maps = _shard_inputs(
        np.asarray(x), np.asarray(w_attn), np.asarray(b_attn), np.asarray(w_proj),
        T, C, dt_mm=DT_MM,
    )
    res = _run(nc, in_maps, trace=_trace)
    if _results_out is not None:
        _results_out.append(res)
    # Host gather: sum the 4 tensor-parallel partials per batch, add the exact
    # V-bias term (softmax rows sum to 1) and the projection bias.
    vbias_term = (
        np.asarray(b_attn)[2 * C:3 * C].astype(np.float32) @ np.asarray(w_proj).T
    )
    out = np.empty((B, T, C), dtype=np.float32)
    for b in range(B):
        acc = res.results[4 * b]["yp"].astype(np.float32).copy()
        for g in range(1, 4):
            acc += res.results[4 * b + g]["yp"]
        out[b] = acc + vbias_term + np.asarray(b_proj)
    return out
